# revision 5
# baseline (speedup 1.0000x reference)
"""Trainium2 Bass kernel: causal attention block with query-axis softmax.

Reference math (per batch element b):
    Q = X @ Wq + bq ; K = X @ Wk + bk ; V = X @ Wv + bv          # [T, D]
    logits[i, j] = Q[i] . K[j],  logits[i, j] = -inf where j > i
    probs = softmax(logits, axis=i) / sqrt(1024)                 # QUERY axis
    out = X + probs @ V

Distribution: pure data-parallel — B=8 batch elements, one per NeuronCore,
weights replicated, no collectives.

Per-core implementation notes:
  * Everything is done in "transposed logit" space LT[j, i] = logits[i, j],
    so the axis-i softmax is a per-partition free-axis reduction.
  * TensorEngine compute in bf16 (1 cycle/row, vs 4 for fp32), PSUM
    accumulation in fp32.  Final residual add + output in fp32.
  * Causal structure: LT row-chunk jc (128 j's) only needs i >= 128*jc, so
    logits/PT rows are stored triangularly and the PT @ V matmuls skip
    fully-masked blocks.
"""

import sys

if "/opt/trn_rl_repo" not in sys.path:
    sys.path.insert(0, "/opt/trn_rl_repo")

import numpy as np

import concourse.bass as bass
import concourse.mybir as mybir
import concourse.tile as tile
from concourse import bacc
from concourse.bass import ts
from concourse.bass_utils import run_bass_kernel_spmd
from concourse.masks import make_identity

B, T, D = 8, 2048, 1024
P = 128
DC = D // P  # 8 feature chunks
TC = T // P  # 16 token chunks
NS = 512  # matmul moving free-dim
SL = T // NS  # 4 slices per full row
F32 = mybir.dt.float32
BF16 = mybir.dt.bfloat16
NEG = -1.0e30
N_CORES = 8


def host_tri_mask() -> np.ndarray:
    """[128, 128] additive mask for the diagonal block of LT row-chunk jc:
    entry [p, c] (j = jc*128+p, i = jc*128+c) is 0 where i >= j else -1e30."""
    p = np.arange(P)[:, None]
    c = np.arange(P)[None, :]
    return np.where(c >= p, 0.0, NEG).astype(np.float32)


def build_nc():
    nc = bacc.Bacc("TRN2", target_bir_lowering=False, debug=False)

    x_d = nc.declare_dram_parameter("minibatch", [T, D], F32, isOutput=False)
    wq_d = nc.declare_dram_parameter("Wq", [D, D], F32, isOutput=False)
    bq_d = nc.declare_dram_parameter("bq", [D], F32, isOutput=False)
    wk_d = nc.declare_dram_parameter("Wk", [D, D], F32, isOutput=False)
    bk_d = nc.declare_dram_parameter("bk", [D], F32, isOutput=False)
    wv_d = nc.declare_dram_parameter("Wv", [D, D], F32, isOutput=False)
    bv_d = nc.declare_dram_parameter("bv", [D], F32, isOutput=False)
    tri_d = nc.declare_dram_parameter("tri_mask", [P, P], F32, isOutput=False)
    out_d = nc.declare_dram_parameter("out", [T, D], F32, isOutput=True)

    with tile.TileContext(nc) as tc:
        with (
            tc.tile_pool(name="persist", bufs=1) as persist,
            tc.tile_pool(name="wpool", bufs=8) as wpool,
            tc.tile_pool(name="fstage", bufs=4) as fstage,
            tc.tile_pool(name="xbfp", bufs=2) as xbfp,
            tc.tile_pool(name="stats", bufs=4) as stats,
            tc.tile_pool(name="psum", bufs=2, space="PSUM") as psum,
        ):
            # ---- constants ----
            ident = persist.tile([P, P], BF16, tag="ident", name="ident")
            make_identity(nc, ident)
            ones = persist.tile([1, NS], BF16, tag="ones", name="ones")
            nc.vector.memset(ones, 1.0)
            trimask = persist.tile([P, P], F32, tag="trimask", name="trimask")
            nc.sync.dma_start(out=trimask, in_=tri_d[:, :])
            b_sb = {}
            for nm, bd in (("q", bq_d), ("k", bk_d), ("v", bv_d)):
                bt = persist.tile([1, D], BF16, tag=f"bias_{nm}", name=f"bias_{nm}")
                nc.gpsimd.dma_start(out=bt, in_=bd[None, :])  # f32 -> bf16 cast
                b_sb[nm] = bt

            # ---- persistent activations (bf16) ----
            XT = persist.tile([P, DC, T], BF16, tag="XT", name="XT")  # X^T [d, i]
            QT = persist.tile([P, DC, T], BF16, tag="QT", name="QT")  # Q^T [k, i]
            KT = persist.tile([P, DC, T], BF16, tag="KT", name="KT")  # K^T [k, j]
            V = persist.tile([P, TC, D], BF16, tag="V", name="V")  # V [j, v]
            # triangular PT rows: row jc holds probs^T[j, i] for i >= 128*jc
            PT = [
                persist.tile([P, T - P * jc], BF16, tag=f"PT{jc}", name=f"PT{jc}")
                for jc in range(TC)
            ]

            # ================= phase A: X^T via PE transpose =================
            for ic in range(TC):
                xf = fstage.tile([P, D], F32, tag="f32stage", bufs=4, name="xf")
                nc.sync.dma_start(out=xf, in_=x_d[ts(ic, P), :])
                xb = xbfp.tile([P, D], BF16, tag="xbf", bufs=2)
                nc.vector.tensor_copy(out=xb, in_=xf)
                pt_ = psum.tile([P, D], BF16, tag="acc", bufs=2)
                for dc in range(DC):
                    nc.tensor.transpose(pt_[:, ts(dc, P)], xb[:, ts(dc, P)], ident)
                nc.scalar.copy(
                    out=XT[:, :, ts(ic, P)],
                    in_=pt_.rearrange("p (dc c) -> p dc c", c=P),
                )

            # ================= phase B: projections =================
            def load_w(w_dram):
                wt = []
                for dc in range(DC):
                    w1 = wpool.tile([P, D], BF16, tag="w", bufs=8)
                    nc.gpsimd.dma_start(out=w1, in_=w_dram[ts(dc, P), :])  # cast
                    wt.append(w1)
                return wt

            # Q^T and K^T: out[k, i] = sum_d W[d, k] * XT[d, i]  (+ bias[k])
            for w_dram, bias, dst in ((wq_d, b_sb["q"], QT), (wk_d, b_sb["k"], KT)):
                wt = load_w(w_dram)
                for m in range(DC):
                    acc = psum.tile([P, T], F32, tag="acc", bufs=2)
                    for dc in range(DC):
                        for s in range(SL):
                            nc.tensor.matmul(
                                acc[:, ts(s, NS)],
                                lhsT=wt[dc][:, ts(m, P)],
                                rhs=XT[:, dc, ts(s, NS)],
                                start=(dc == 0),
                                stop=False,
                            )
                    for s in range(SL):
                        nc.tensor.matmul(
                            acc[:, ts(s, NS)],
                            lhsT=bias[:, ts(m, P)],
                            rhs=ones[:, :],
                            start=False,
                            stop=True,
                        )
                    nc.scalar.copy(out=dst[:, m, :], in_=acc)

            # V: out[j, v] = sum_d XT[d, j] * W[d, v]  (+ bias[v])
            wt = load_w(wv_d)
            for jc in range(TC):
                acc = psum.tile([P, T], F32, tag="acc", bufs=2)
                for dc in range(DC):
                    for vs in range(2):
                        nc.tensor.matmul(
                            acc[:, ts(vs, NS)],
                            lhsT=XT[:, dc, ts(jc, P)],
                            rhs=wt[dc][:, ts(vs, NS)],
                            start=(dc == 0),
                            stop=False,
                        )
                for vs in range(2):
                    nc.tensor.matmul(
                        acc[:, ts(vs, NS)],
                        lhsT=ones[:, 0:P],
                        rhs=b_sb["v"][:, ts(vs, NS)],
                        start=False,
                        stop=True,
                    )
                nc.vector.tensor_copy(out=V[:, jc, :], in_=acc[:, 0:D])

            # ============ phase C+D: logits^T rows + query-axis softmax ============
            # LT[j, i] = sum_k KT[k, j] QT[k, i], computed for i-slice groups
            # s >= jc//4; softmax over free axis i per partition row j.
            for jc in range(TC):
                g, r = jc // 4, jc % 4
                off = NS * g  # i-offset of first computed column
                L = T - off  # computed row length
                dstart = P * r  # diagonal block offset within computed region
                acc = psum.tile([P, T], F32, tag="acc", bufs=2)
                for kc in range(DC):
                    for s in range(g, SL):
                        nc.tensor.matmul(
                            acc[:, s * NS - off : (s + 1) * NS - off],
                            lhsT=KT[:, kc, ts(jc, P)],
                            rhs=QT[:, kc, ts(s, NS)],
                            start=(kc == 0),
                            stop=(kc == DC - 1),
                        )
                # causal mask on the diagonal 128-block; columns left of it
                # are never read, columns right of it are fully valid.
                nc.vector.tensor_add(
                    out=acc[:, dstart : dstart + P],
                    in0=acc[:, dstart : dstart + P],
                    in1=trimask,
                )
                valid = acc[:, dstart:L]
                negmax = stats.tile([P, 1], F32, tag="negmax", bufs=4)
                nc.vector.reduce_max(
                    out=negmax, in_=valid, axis=mybir.AxisListType.X, negate=True
                )
                ssum = stats.tile([P, 1], F32, tag="ssum", bufs=4)
                nc.scalar.activation(
                    out=PT[jc][:, :],
                    in_=valid,
                    func=mybir.ActivationFunctionType.Exp,
                    bias=negmax,
                    scale=1.0,
                    accum_out=ssum,
                )
                rv = stats.tile([P, 1], F32, tag="rv", bufs=4)
                nc.vector.reciprocal(out=rv, in_=ssum)
                # fold per-j softmax denominator and the 1/sqrt(1024) scale
                # into V's rows: V[j, :] *= rv[j] / 32
                nc.vector.tensor_scalar(
                    out=V[:, jc, :],
                    in0=V[:, jc, :],
                    scalar1=rv,
                    scalar2=1.0 / 32.0,
                    op0=mybir.AluOpType.mult,
                    op1=mybir.AluOpType.mult,
                )

            # ============ phase E: read = probs @ V, residual, store ============
            # read[i, v] = sum_j PT[j, i] * V[j, v]; block (ic, jc) is all-zero
            # for jc > ic (causality) and skipped.
            for ic in range(TC):
                acc = psum.tile([P, T], F32, tag="acc", bufs=2)
                for jc in range(ic + 1):
                    blk = PT[jc][:, (ic - jc) * P : (ic - jc + 1) * P]
                    for vs in range(2):
                        nc.tensor.matmul(
                            acc[:, ts(vs, NS)],
                            lhsT=blk,
                            rhs=V[:, jc, ts(vs, NS)],
                            start=(jc == 0),
                            stop=(jc == ic),
                        )
                xf = fstage.tile([P, D], F32, tag="f32stage", bufs=4, name="xf")
                nc.sync.dma_start(out=xf, in_=x_d[ts(ic, P), :])
                ot = fstage.tile([P, D], F32, tag="f32stage", bufs=4, name="ot")
                nc.vector.tensor_add(out=ot, in0=acc[:, 0:D], in1=xf)
                nc.sync.dma_start(out=out_d[ts(ic, P), :], in_=ot)

    nc.finalize()
    return nc


_NC_CACHE = None


def get_nc():
    global _NC_CACHE
    if _NC_CACHE is None:
        _NC_CACHE = build_nc()
    return _NC_CACHE


def make_in_maps(inputs: dict) -> list[dict]:
    mb = np.ascontiguousarray(np.asarray(inputs["minibatch"], dtype=np.float32))
    assert mb.shape == (B, T, D)
    shared = {
        k: np.ascontiguousarray(np.asarray(inputs[k], dtype=np.float32))
        for k in ("Wq", "bq", "Wk", "bk", "Wv", "bv")
    }
    shared["tri_mask"] = host_tri_mask()
    return [{"minibatch": mb[c], **shared} for c in range(N_CORES)]


def kernel(**inputs) -> np.ndarray:
    nc = get_nc()
    in_maps = make_in_maps(inputs)
    res = run_bass_kernel_spmd(nc, in_maps, core_ids=list(range(N_CORES)))
    return np.stack([res.results[c]["out"] for c in range(N_CORES)], axis=0)


if __name__ == "__main__":
    rng = np.random.default_rng(0)
    demo = {
        "minibatch": rng.standard_normal((B, T, D), dtype=np.float32),
        "Wq": rng.standard_normal((D, D), dtype=np.float32) * 0.02,
        "bq": np.zeros(D, np.float32),
        "Wk": rng.standard_normal((D, D), dtype=np.float32) * 0.02,
        "bk": np.zeros(D, np.float32),
        "Wv": rng.standard_normal((D, D), dtype=np.float32) * 0.02,
        "bv": np.zeros(D, np.float32),
    }
    out = kernel(**demo)
    print(out.shape, out.dtype)
